# revision 36
# baseline (speedup 1.0000x reference)
"""Trainium2 Bass kernel for nn_Caps2dMatwo (capsule conv + dual routing).

Strategy (8 NeuronCores, no cross-core communication):
  - Shard: core k handles batch n=k//4, H-rows [32*(k%4), 32*(k%4)+32).
  - PE stage: the 3x3 conv and the capsule pose/appearance transforms are
    fused into 9 per-tap matmuls over a permuted 128-channel axis
    (i,c,q,j), block-diagonal per 32-row i-block (4 row-tiled matmuls per
    tap), plus a ones-row matmul that injects the appearance biases so no
    vector-engine bias add is needed.  The PSUM output holds, per pixel:
    u_hat (t,c,pa,i,k), the iteration-1 routing input p1 = 0.5*sum_c
    u_hat, and raw j=3 conv taps for the coordinate-addition fixup.
  - Routing (3 iterations, sigmoid coupling, psquash/matwo_squash) runs
    batched over G=8 rows per instruction: pixels on partitions, (row,
    capsule dims) on the free axis.  The squash factors are never applied
    to a materialized v tensor: the agreement update uses
    b += ff * (sum_z uh_pos*p_pos) * (sum_z uh_app*p_app) with
    ff = f_pos*gf folded into one small multiply.
  - Engine budget: broadcast-operand muls run on DVE (f16 2x mode);
    z/c reductions are per-engine complete add-trees split 5 rows DVE /
    3 rows Pool (2x trees beat 1x tensor_reduce ~2:1); PSUM extraction,
    squares and sigmoids run on ACT (Square/Sigmoid share one ACT table).
  - Output stays pixel-major ([w, row, t, pa, z]) in one contiguous DMA
    per group; the host does the final transpose (device transposes cost
    ~80us of DMA-transpose in the old layout).
"""
import os
from contextlib import ExitStack

import numpy as np

# problem constants (hardcoded per spec)
N, T0, T1 = 2, 4, 8
H, W = 128, 128
PD, AD = 4, 4
Z = 32
NBLK = 360          # psum cols per i-block: 256 own + 64 usum + 32 craw + 8 csum
ROWS = 32           # output rows per core
P = 128
G = 8               # rows per batched routing group

_CACHE = {}


# ----------------------------------------------------------------- host prep
def _build_weights(W_conv, W_pos, W_app, b_app):
    """W_eff for the fused conv+pose matmul.

    Returns:
      w_in    [128, 9, 360]  per-partition weights, partition = i*32+c*8+q*4+j
      bias_uh [4c, 8t, 4k]   appearance bias term  b_app * colsum(Mapp)
      bias_p1 [8t, 4k]       0.5 * sum_c bias_uh
    """
    Kc = np.asarray(W_conv, np.float64)[:, :, :, 0, :]          # [c,dy,dx,t1]
    Mpos = np.asarray(W_pos, np.float64).reshape(T0, T1, PD, PD).copy()
    Mpos = Mpos / np.sqrt(np.maximum((Mpos ** 2).sum(axis=2, keepdims=True), 1e-12))
    Mapp = np.asarray(W_app, np.float64).reshape(T0, T1, AD, AD)
    Sapp = Mapp.sum(axis=2)                                      # [c,t,k]

    W_eff = np.zeros((9, 128, 4, NBLK), np.float64)
    for tap in range(9):
        dy, dx = tap // 3, tap % 3
        for i in range(4):
            for c in range(4):
                for q in range(2):
                    for j in range(4):
                        row = i * 32 + c * 8 + q * 4 + j
                        for t in range(q, 8, 2):
                            kpos = Kc[c, dy, dx, t // 2]
                            kapp = Kc[c, dy, dx, 4 + t // 2]
                            base = t * 32 + c * 8
                            W_eff[tap, row, i, base:base + 4] = kpos * Mpos[c, t, j]
                            W_eff[tap, row, i, base + 4:base + 8] = kapp * Mapp[c, t, j]
                            ub = 256 + t * 8
                            W_eff[tap, row, i, ub:ub + 4] += 0.5 * kpos * Mpos[c, t, j]
                            W_eff[tap, row, i, ub + 4:ub + 8] += 0.5 * kapp * Mapp[c, t, j]
                            if j == 3:
                                W_eff[tap, row, i, 320 + t * 4 + c] = kpos
                                W_eff[tap, row, i, 352 + t] += 0.5 * kpos
    # [9, 128, 4, 360] -> [128, 9, 360] picking each partition's own block
    w_in = np.zeros((128, 9, NBLK), np.float32)
    for i in range(4):
        w_in[i * 32:(i + 1) * 32] = W_eff[:, i * 32:(i + 1) * 32, i, :].transpose(1, 0, 2)
    bias_uh = np.einsum('ct,ctk->ctk', np.asarray(b_app, np.float64), Sapp)
    bias_p1 = 0.5 * bias_uh.sum(axis=0)
    return w_in, bias_uh.astype(np.float32), bias_p1.astype(np.float32)


def _shard_x(x):
    """x [N,T0,Z,H,W] -> list of 8 arrays [128, 34*130] (permuted channels)."""
    xp = np.zeros((N, T0, Z, H + 2, W + 2), np.float32)
    xp[:, :, :, 1:H + 1, 1:W + 1] = np.asarray(x, np.float32)
    # z = q*16 + i*4 + j ; partition = i*32 + c*8 + q*4 + j
    xq = xp.reshape(N, T0, 2, 4, 4, H + 2, W + 2)                # n c q i j h w
    xperm = np.ascontiguousarray(xq.transpose(0, 3, 1, 2, 4, 5, 6)
                                 ).reshape(N, 128, H + 2, W + 2)
    shards = []
    for core in range(8):
        n, rb = core // 4, (core % 4) * 32
        shards.append(np.ascontiguousarray(
            xperm[n, :, rb:rb + 34, :]).reshape(128, 34 * 130))
    return shards


# ------------------------------------------------------------- bass module
def _build_module():
    import concourse.bass as bass
    import concourse.tile as tile
    import concourse.mybir as mybir
    from concourse import bacc

    f32 = mybir.dt.float32
    f16 = mybir.dt.float16
    AX = mybir.AxisListType.X
    OP = mybir.AluOpType
    AF = mybir.ActivationFunctionType

    nc = bacc.Bacc("TRN2", num_devices=8, debug=False)
    x_d = nc.dram_tensor("x_shard", [128, 34 * 130], f16, kind="ExternalInput").ap()
    w_d = nc.dram_tensor("w_eff", [128, 9, NBLK], f16, kind="ExternalInput").ap()
    buh_d = nc.dram_tensor("bias_uh", [128, 512], f16, kind="ExternalInput").ap()
    bp1_d = nc.dram_tensor("bias_p1", [128, 128], f16, kind="ExternalInput").ap()
    cxy_d = nc.dram_tensor("cxy", [128, ROWS * 2], f32, kind="ExternalInput").ap()
    # pixel-major output [w-pixel, (row, t, pa, z)] — host transposes
    out_d = nc.dram_tensor("out_shard", [128, ROWS * 256], f16,
                           kind="ExternalOutput").ap()

    NGRP = ROWS // G

    with tile.TileContext(nc) as tc, ExitStack() as ctx:
        const = ctx.enter_context(tc.tile_pool(name="const", bufs=1))
        grp = ctx.enter_context(tc.tile_pool(name="grp", bufs=2))
        work = ctx.enter_context(tc.tile_pool(name="work", bufs=2))
        psum = ctx.enter_context(tc.tile_pool(name="psum", bufs=2, space="PSUM"))

        x_sb = const.tile([P, 34, 130], f16)
        nc.sync.dma_start(out=x_sb[:].rearrange("p a b -> p (a b)"), in_=x_d)
        w_sb = const.tile([P, 9, NBLK], f16)
        nc.sync.dma_start(out=w_sb, in_=w_d)
        buh = const.tile([P, 8, 4, 16], f16)      # (t, c, (i k)) app bias
        nc.sync.dma_start(out=buh[:].rearrange("p a b c -> p (a b c)"), in_=buh_d)
        bp1 = const.tile([P, 8, 16], f16)         # (t, (i k)) p1 app bias
        nc.sync.dma_start(out=bp1[:].rearrange("p a b -> p (a b)"), in_=bp1_d)
        cxy = const.tile([P, ROWS, 2], f32)       # per row: (w/128, h/128)
        nc.sync.dma_start(out=cxy[:].rearrange("p a b -> p (a b)"), in_=cxy_d)
        eps_t = const.tile([P, 1], f32)
        nc.vector.memset(eps_t, 1e-9)
        vout = const.tile([P, ROWS, 8, 2, 16], f16)  # (r, t, pa, z) output staging

        st = {}  # per-group live tiles
        # the big contiguous tree-add level is row-split DVE/Pool; all
        # broadcast-operand muls stay on DVE (2x f16 mode)
        TREE = [(lambda: nc.vector, slice(0, G), G)]

        def s0_matmul_copy(gi):
            g0 = gi * G
            uh = grp.tile([P, G, 2, 8, 4, 16], f16, tag="uh")   # (g, pa, t, c, ik)
            p1 = grp.tile([P, G, 2, 8, 16], f16, tag="p1")      # (g, pa, t, ik)
            craw = grp.tile([P, G, 8, 4, 4], f16, tag="craw")   # (g, t, c, i)
            csum = grp.tile([P, G, 8, 4], f16, tag="csum")      # (g, t, i)
            st[gi] = dict(uh=uh, p1=p1, craw=craw, csum=csum)
            for j in range(G):
                r = g0 + j
                ups = psum.tile([P, 2048], f32, tag="ups")
                for tap in range(9):
                    dy, dx = tap // 3, tap % 3
                    patch = x_sb[:, r + dy, dx:dx + 128]
                    for i in range(4):
                        nc.tensor.matmul(
                            ups[:, i * 512:i * 512 + 360],
                            lhsT=patch[32 * i:32 * (i + 1), :],
                            rhs=w_sb[32 * i:32 * (i + 1), tap, :],
                            start=(tap == 0), stop=(tap == 8),
                            tile_position=(32 * i, 0))
                upsr = ups[:].rearrange("p (i n) -> p i n", i=4)
                # uh own block + p1, one copy per pa (ISA caps APs at 3 free dims)
                src_uh = upsr[:, :, 0:256].rearrange(
                    "p i (tc pa k) -> p i tc pa k", tc=32, pa=2)
                src_p1 = upsr[:, :, 256:320].rearrange(
                    "p i (t pa k) -> p i t pa k", t=8, pa=2)
                for pa in (0, 1):
                    nc.scalar.copy(
                        uh[:, j, pa].rearrange("p t c (i k) -> p i (t c) k", i=4),
                        src_uh[:, :, :, pa])
                    nc.scalar.copy(
                        p1[:, j, pa].rearrange("p t (i k) -> p i t k", i=4),
                        src_p1[:, :, :, pa])
                # craw: iterate (i, tc); csum: iterate (i, t)
                nc.scalar.copy(craw[:, j].rearrange("p t c i -> p i (t c)"),
                               upsr[:, :, 320:352])
                nc.scalar.copy(csum[:, j].rearrange("p t i -> p i t"),
                               upsr[:, :, 352:360])

        def s1_fix(gi):
            g0 = gi * G
            uh, p1 = st[gi]["uh"], st[gi]["p1"]
            craw, csum = st[gi]["craw"], st[gi]["csum"]
            # coordinate addition (all DVE; Pool reads broadcast
            # operands very slowly)
            tmp = work.tile([P, G, 128, 2], f16, tag="tmp")
            for eng, gs, ng in [(lambda: nc.vector, slice(0, G), G)]:
                nc_e = eng()
                uha = uh[:, gs, 1].rearrange("p g t c z -> p g (t c z)")
                nc_e.tensor_add(uha, uha,
                                buh[:].rearrange("p t c z -> p (t c z)")
                                .unsqueeze(1).broadcast_to((P, ng, 512)))
                p1a = p1[:, gs, 1].rearrange("p g t z -> p g (t z)")
                nc_e.tensor_add(p1a, p1a,
                                bp1[:].rearrange("p t z -> p (t z)")
                                .unsqueeze(1).broadcast_to((P, ng, 128)))
                crawf = craw[:, gs].rearrange("p g t c i -> p g (t c i)")
                nc_e.tensor_scalar_mul(tmp[:, gs, :, 0], crawf, cxy[:, 0, 0:1])
                nc_e.tensor_mul(
                    tmp[:, gs, :, 1], crawf,
                    cxy[:, g0 + gs.start:g0 + gs.stop, 1].unsqueeze(2)
                    .broadcast_to((P, ng, 128)))
                uv = uh[:, gs, 0].rearrange(
                    "p g t c (i k) -> p g (t c i) k", i=4)[:, :, :, 0:2]
                nc_e.tensor_add(uv, uv, tmp[:, gs])
            csumf = csum[:].rearrange("p g t i -> p g (t i)")
            tmpp = work.tile([P, G, 32, 2], f16, tag="tmpp")
            nc.vector.tensor_scalar_mul(tmpp[:, :, :, 0], csumf, cxy[:, 0, 0:1])
            nc.vector.tensor_mul(
                tmpp[:, :, :, 1], csumf,
                cxy[:, g0:g0 + G, 1].unsqueeze(2).broadcast_to((P, G, 32)))
            pv = p1[:, :, 0].rearrange(
                "p g t (i k) -> p g (t i) k", i=4)[:, :, :, 0:2]
            nc.vector.tensor_add(pv, pv, tmpp[:])

        def squash(gi, p_ap, ff_out, v_ap):
            """Squash factors facs=[1/mx | gf] for p=[p_pos|p_app].

            ff_out [P,G,8]: product f_pos*gf for the b-update fold.
            v_ap: if set (final iter), writes v3 [g, t, pa, z] into vout."""
            md = work.tile([P, G, 2, 8], f32, tag="md")   # [mx | den]
            sq = work.tile([P, G, 8, 16], f16, tag="sq")
            s = work.tile([P, G, 8], f32, tag="s")
            nc.vector.tensor_reduce(out=md[:, :, 0], in_=p_ap[:, :, 0], axis=AX,
                                    op=OP.max, apply_absolute_value=True)
            nc.scalar.activation(sq[:], p_ap[:, :, 1], AF.Square)
            # 2x-mode add-tree over z then a short 1x reduce (f16 partials)
            nc.vector.tensor_add(sq[:, :, :, 0:8], sq[:, :, :, 0:8],
                                 sq[:, :, :, 8:16])
            nc.vector.tensor_add(sq[:, :, :, 0:4], sq[:, :, :, 0:4],
                                 sq[:, :, :, 4:8])
            nc.vector.tensor_add(sq[:, :, :, 0:2], sq[:, :, :, 0:2],
                                 sq[:, :, :, 2:4])
            with nc.allow_low_precision("f16 partials, f32 total"):
                nc.vector.tensor_add(s.unsqueeze(3) if False else s,
                                     sq[:, :, :, 0], sq[:, :, :, 1])
            sq1 = work.tile([P, G, 8], f32, tag="sq1")
            nc.scalar.activation(sq1, s, AF.Sqrt, bias=eps_t[:, 0:1])
            nc.vector.scalar_tensor_tensor(out=md[:, :, 1], in0=s, scalar=1.0,
                                           in1=sq1, op0=OP.add, op1=OP.mult)
            facs = work.tile([P, G, 2, 8], f32, tag="facs")   # [rmx | gf]
            nc.vector.reciprocal_approx_fast(
                facs[:].rearrange("p g pa t -> p (g pa t)"),
                md[:].rearrange("p g pa t -> p (g pa t)"))
            nc.vector.tensor_mul(facs[:, :, 1], s, facs[:, :, 1])
            if ff_out is not None:
                nc.vector.tensor_mul(ff_out, facs[:, :, 0], facs[:, :, 1])
            if v_ap is not None:
                # final squash: write v3 [g, t, pa, z] channel-major, per pa.
                for pa in (0, 1):
                    nc.vector.tensor_mul(
                        v_ap[:, :, :, pa, :], p_ap[:, :, pa],
                        facs[:, :, pa].unsqueeze(3)
                        .broadcast_to((P, G, 8, 16)))

        def rout_badd(gi, p_ap, ff, first):
            """b += ff * (sum_z uh_pos*p_pos) * (sum_z uh_app*p_app).

            Squash factors are folded in via ff = f_pos*gf (per g,t), so the
            v tensors are never materialized."""
            uh = st[gi]["uh"]
            wp = work.tile([P, G, 16, 4, 16], f16, tag="wp")
            wb = work.tile([P, G, 16, 4, 8], f16, tag="wb")
            ab = work.tile([P, G, 2, 8, 4], f16, tag="ab")
            # wp = uh * p (p broadcast over c, mid-dim: 2x-eligible) on DVE
            uhm = uh[:].rearrange("p g pa t c z -> p (g pa t) c z")
            pm = (p_ap[:].rearrange("p g pa t z -> p (g pa t) z")
                  .unsqueeze(2).broadcast_to((P, G * 16, 4, 16)))
            wpf = wp[:].rearrange("p g x c z -> p (g x) c z")
            nc.vector.tensor_mul(wpf, uhm, pm)
            # z-tree: complete per-engine trees over row slices, so DVE and
            # Pool run independently until the final b combine
            for eng, gs, ng in TREE:
                nc_e = eng()
                wps = wp[:, gs].rearrange("p g x c z -> p (g x) c z")
                wbs = wb[:, gs].rearrange("p g x c k -> p (g x) c k")
                nc_e.tensor_add(wbs, wps[:, :, :, 0:8], wps[:, :, :, 8:16])
                nc_e.tensor_add(wbs[:, :, :, 0:4], wbs[:, :, :, 0:4],
                                wbs[:, :, :, 4:8])
                nc_e.tensor_add(wbs[:, :, :, 0:2], wbs[:, :, :, 0:2],
                                wbs[:, :, :, 2:4])
                nc_e.tensor_add(
                    ab[:, gs].rearrange("p g pa t c -> p (g pa t) c"),
                    wbs[:, :, :, 0], wbs[:, :, :, 1])
            bt = work.tile([P, G, 8, 4], f16, tag="bt")
            nc.vector.tensor_mul(bt, ab[:, :, 0], ab[:, :, 1])
            ffb = ff[:].unsqueeze(3).broadcast_to((P, G, 8, 4))
            if first:
                b = work.tile([P, G, 8, 4], f32, tag="b")
                st[gi]["b"] = b
                nc.vector.tensor_mul(b, bt[:], ffb)
            else:
                b = st[gi]["b"]
                rt = work.tile([P, G, 8, 4], f32, tag="rt")
                nc.vector.tensor_mul(rt, bt[:], ffb)
                nc.vector.tensor_add(b, b[:], rt[:])

        def sig_p(gi, p_out):
            uh, b = st[gi]["uh"], st[gi]["b"]
            rd = work.tile([P, G, 2, 8, 4], f16, tag="rd")
            nc.scalar.activation(rd[:, :, 0], b, AF.Sigmoid)
            nc.scalar.activation(rd[:, :, 1], b, AF.Sigmoid)
            m = work.tile([P, G, 16, 4, 16], f16, tag="wp")
            ta = work.tile([P, G, 16, 2, 16], f16, tag="ta")
            nc.vector.tensor_mul(
                m[:].rearrange("p g x c z -> p (g x) c z"),
                uh[:].rearrange("p g pa t c z -> p (g pa t) c z"),
                rd[:].rearrange("p g pa t c -> p (g pa t) c")
                .unsqueeze(3).broadcast_to((P, G * 16, 4, 16)))
            for eng, gs, ng in TREE:
                nc_e = eng()
                ms = m[:, gs].rearrange("p g x c z -> p (g x) c z")
                tas = ta[:, gs].rearrange("p g x c z -> p (g x) c z")
                nc_e.tensor_add(tas, ms[:, :, 0:2, :], ms[:, :, 2:4, :])
                nc_e.tensor_add(
                    p_out[:, gs].rearrange("p g pa t z -> p (g pa t) z"),
                    tas[:, :, 0, :], tas[:, :, 1, :])

        def s2_routing(gi):
            g0 = gi * G
            p1 = st[gi]["p1"]
            ff1 = work.tile([P, G, 8], f16, tag="ff")
            squash(gi, p1[:], ff1, None)
            rout_badd(gi, p1[:], ff1, first=True)
            p2 = work.tile([P, G, 2, 8, 16], f16, tag="p2")
            sig_p(gi, p2)
            ff2 = work.tile([P, G, 8], f16, tag="ff")
            squash(gi, p2[:], ff2, None)
            rout_badd(gi, p2[:], ff2, first=False)
            p3 = work.tile([P, G, 2, 8, 16], f16, tag="p2")
            sig_p(gi, p3)
            squash(gi, p3[:], None, vout[:, g0:g0 + G])
            nc.sync.dma_start(
                out=out_d[:, g0 * 256:(g0 + G) * 256],
                in_=vout[:, g0:g0 + G].rearrange("p a b c d -> p (a b c d)"))
            del st[gi]

        for gi in range(NGRP):
            s0_matmul_copy(gi)
            s1_fix(gi)
            s2_routing(gi)

    nc.compile()
    return nc


def _make_in_map(core, shards, w_in, bias_uh, bias_p1):
    """Per-core input dict. bias_uh [c,t,k] and bias_p1 [t,k] get i-expanded."""
    rb = (core % 4) * 32
    # (t, c, (i, k)) with i broadcast
    buh_in = np.broadcast_to(
        bias_uh.transpose(1, 0, 2)[:, :, None, :], (8, 4, 4, 4)).reshape(1, 512)
    buh_in = np.broadcast_to(buh_in, (128, 512)).copy()
    bp1_in = np.broadcast_to(
        bias_p1[:, None, :], (8, 4, 4)).reshape(1, 128)
    bp1_in = np.broadcast_to(bp1_in, (128, 128)).copy()
    cxy_in = np.zeros((128, ROWS, 2), np.float32)
    cxy_in[:, :, 0] = (np.arange(128, dtype=np.float32) / 128.0)[:, None]
    cxy_in[:, :, 1] = ((rb + np.arange(ROWS, dtype=np.float32)) / 128.0)[None, :]
    return {
        "x_shard": shards[core].astype(np.float16),
        "w_eff": w_in.astype(np.float16),
        "bias_uh": buh_in.astype(np.float16),
        "bias_p1": bp1_in.astype(np.float16),
        "cxy": cxy_in.reshape(128, ROWS * 2),
    }


def kernel(x, W_conv, W_pos, W_app, b_app):
    from concourse.bass_utils import run_bass_kernel_spmd

    if "nc" not in _CACHE:
        _CACHE["nc"] = _build_module()
    nc = _CACHE["nc"]

    w_in, bias_uh, bias_p1 = _build_weights(W_conv, W_pos, W_app, b_app)
    shards = _shard_x(x)
    in_maps = [_make_in_map(core, shards, w_in, bias_uh, bias_p1)
               for core in range(8)]

    trace = bool(int(os.environ.get("CAPS_TRACE", "0")))
    res = run_bass_kernel_spmd(nc, in_maps, core_ids=list(range(8)), trace=trace)
    _CACHE["last_result"] = res

    out = np.zeros((N, T1, Z, H, W), np.float32)
    for core in range(8):
        n, rb = core // 4, (core % 4) * 32
        # [w, r, t, pa, zz] -> [t, (pa zz), r, w]
        o = res.results[core]["out_shard"].astype(np.float32).reshape(
            128, ROWS, 8, 2, 16).transpose(2, 3, 4, 1, 0).reshape(
            8, 32, ROWS, 128)
        out[n, :, :, rb:rb + 32, :] = o
    return out

